# revision 14
# baseline (speedup 1.0000x reference)
"""MultiHeadAttention Trainium2 kernel (8 NeuronCores, SPMD).

Sharding: core c = (batch b=c//4, head-group g=c%4); each core owns 4 of 16
heads for one batch element. Wq/Wk/Wv are split by output features (tensor
parallel on heads), Wo by input features (row parallel); the 4 partial
[S, D] outputs per batch are summed on the host.

v2 structure (vs v1): the attention kt-loop is software-pipelined with a
lag-2 ctx stage so the PE never waits in-line for the ACT exp of the same
kt (scores(kt+2) issues before ctx(kt)).  The softmax-normalization
epilogue avoids the DRAM bounce: odd heads accumulate ctx^T at partition
offset 63 via an [ones|dims] V augmentation, so the per-head reciprocal
rows broadcast with gpsimd.partition_broadcast and the normalized ct
assembles in place across partitions 0:64 / 64:128.  ACT runs exp only;
all other PSUM->SBUF traffic sits on Pool (gpsimd) / DVE.
"""

import numpy as np

B, S, D = 2, 2048, 1024
H, DK = 16, 64
HG = 4                 # heads per core
FC = HG * DK           # 256 features per core
NCORES = 8
P = 128
KSUB = D // P          # 8 contraction subtiles for projections
FT = FC // P           # 2 feature tiles (= head pairs)
NKT = S // P           # 16 key-position tiles
QC = 512               # q-chunk size
NQC = S // QC          # 4
SCHUNK = 512           # s-chunk for streaming projections
NSC = S // SCHUNK      # 4
LAG = 3                # ctx lags scores by LAG kt-steps

_PROGRAM = None        # cached Bass program - build once per process


def _build_program():
    from contextlib import ExitStack

    import concourse.bass as bass
    import concourse.mybir as mybir
    import concourse.tile as tile
    from concourse import bacc

    f32 = mybir.dt.float32
    bf16 = mybir.dt.bfloat16
    EXP = mybir.ActivationFunctionType.Exp

    nc = bacc.Bacc("TRN2", target_bir_lowering=False, debug=False)

    qT = nc.dram_tensor("qT", [D, S], bf16, kind="ExternalInput")
    kT = nc.dram_tensor("kT", [D, S], bf16, kind="ExternalInput")
    vT = nc.dram_tensor("vT", [D, S], bf16, kind="ExternalInput")
    wqT = nc.dram_tensor("wqT", [D, FC], bf16, kind="ExternalInput")
    wkT = nc.dram_tensor("wkT", [D, FC], bf16, kind="ExternalInput")
    wvT = nc.dram_tensor("wvT", [D, FC], bf16, kind="ExternalInput")
    woT = nc.dram_tensor("woT", [FC, D], bf16, kind="ExternalInput")
    bq = nc.dram_tensor("bq", [FC], f32, kind="ExternalInput")
    bk = nc.dram_tensor("bk", [FC], f32, kind="ExternalInput")
    out = nc.dram_tensor("out", [S, D], f32, kind="ExternalOutput")

    with tile.TileContext(nc) as tc, ExitStack() as ctx, nc.allow_low_precision(
        reason="bf16 matmul operands are intentional"
    ):
        weights = ctx.enter_context(tc.tile_pool(name="weights", bufs=1))
        instream = ctx.enter_context(tc.tile_pool(name="instream", bufs=5))
        persist = ctx.enter_context(tc.tile_pool(name="persist", bufs=1))
        exps = ctx.enter_context(tc.tile_pool(name="exps", bufs=8))
        ctpool = ctx.enter_context(tc.tile_pool(name="ctpool", bufs=2))
        bcast = ctx.enter_context(tc.tile_pool(name="bcast", bufs=2))
        small = ctx.enter_context(tc.tile_pool(name="small", bufs=2))
        outsb = ctx.enter_context(tc.tile_pool(name="outsb", bufs=2))
        ps_sc = ctx.enter_context(tc.tile_pool(name="ps_sc", bufs=2, space="PSUM"))
        ps_acc = ctx.enter_context(tc.tile_pool(name="ps_acc", bufs=4, space="PSUM"))

        # ---- persistent weights ----
        wq_sb = weights.tile([P, KSUB, FC], bf16, tag="wq")
        nc.sync.dma_start(wq_sb, wqT[:, :].rearrange("(o p) f -> p o f", p=P))
        wk_sb = weights.tile([P, KSUB, FC], bf16, tag="wk")
        nc.sync.dma_start(wk_sb, wkT[:, :].rearrange("(o p) f -> p o f", p=P))
        wv_sb = weights.tile([P, KSUB, FC], bf16, tag="wv")
        nc.sync.dma_start(wv_sb, wvT[:, :].rearrange("(o p) f -> p o f", p=P))
        # per-head row blocks of Wo^T on partitions 0:64 (split-K out-proj)
        wo_sb = weights.tile([64, HG, D], bf16, tag="wo")
        nc.sync.dma_start(wo_sb, woT[:, :].rearrange("(h p) j -> p h j", p=64))
        ones_sb = weights.tile([P, 64], bf16, tag="ones")
        nc.gpsimd.memset(ones_sb, 1.0)
        bq_sb = weights.tile([P, FT], f32, tag="bq")
        nc.sync.dma_start(bq_sb, bq[:].rearrange("(t p) -> p t", p=P))
        bk_sb = weights.tile([P, FT], f32, tag="bk")
        nc.sync.dma_start(bk_sb, bk[:].rearrange("(t p) -> p t", p=P))

        # ---- persistent activations ----
        QT = persist.tile([P, FT, S], bf16, tag="QT")   # [feat, seq]
        KT = persist.tile([P, FT, S], bf16, tag="KT")   # [feat, seq]
        # V: [key, kt, h, 66] = [dims(0:64) | ones(64) | pad]; the ones
        # column accumulates the softmax denominator on ctx row 64.
        V = persist.tile([P, NKT, HG, 66], bf16, tag="V")
        nc.gpsimd.memset(V[:, :, :, 64:65], 1.0)

        # ---- K/V projections, streamed over s-chunks; Q projections last ----
        qTr = qT[:, :].rearrange("(o p) s -> p o s", p=P)
        kTr = kT[:, :].rearrange("(o p) s -> p o s", p=P)
        vTr = vT[:, :].rearrange("(o p) s -> p o s", p=P)
        for c in range(NSC):
            sl = slice(c * SCHUNK, (c + 1) * SCHUNK)
            kc = instream.tile([P, KSUB, SCHUNK], bf16, tag="instream")
            nc.sync.dma_start(kc, kTr[:, :, sl])
            for ft in range(FT):
                ps = ps_acc.tile([P, SCHUNK], f32, tag="acc", name="kps")
                for ks in range(KSUB):
                    nc.tensor.matmul(
                        ps,
                        lhsT=wk_sb[:, ks, ft * P:(ft + 1) * P],
                        rhs=kc[:, ks, :],
                        start=(ks == 0),
                        stop=(ks == KSUB - 1),
                    )
                nc.vector.tensor_scalar_add(KT[:, ft, sl], ps, bk_sb[:, ft:ft + 1])
            vc = instream.tile([P, KSUB, SCHUNK], bf16, tag="instream")
            nc.sync.dma_start(vc, vTr[:, :, sl])
            for st in range(SCHUNK // P):
                ps = ps_acc.tile([P, SCHUNK], f32, tag="acc", name="vps")
                for ks in range(KSUB):
                    nc.tensor.matmul(
                        ps[:, :FC],
                        lhsT=vc[:, ks, st * P:(st + 1) * P],
                        rhs=wv_sb[:, ks, :],
                        start=(ks == 0),
                        stop=(ks == KSUB - 1),
                    )
                kt_idx = c * (SCHUNK // P) + st
                nc.vector.tensor_copy(
                    out=V[:, kt_idx, :, 0:DK],
                    in_=ps[:, 0:FC].rearrange("p (h d) -> p h d", h=HG),
                )
        for c in range(NSC):
            sl = slice(c * SCHUNK, (c + 1) * SCHUNK)
            qc_t = instream.tile([P, KSUB, SCHUNK], bf16, tag="instream")
            nc.sync.dma_start(qc_t, qTr[:, :, sl])
            for ft in range(FT):
                ps = ps_acc.tile([P, SCHUNK], f32, tag="acc", name="qps")
                for ks in range(KSUB):
                    nc.tensor.matmul(
                        ps,
                        lhsT=wq_sb[:, ks, ft * P:(ft + 1) * P],
                        rhs=qc_t[:, ks, :],
                        start=(ks == 0),
                        stop=(ks == KSUB - 1),
                    )
                nc.vector.tensor_scalar_add(QT[:, ft, sl], ps, bq_sb[:, ft:ft + 1])

        # ---- attention + output projection: lag-LAG software pipeline ----
        # per (qc, kt) step: emit scores+exp; ctx trails LAG steps behind;
        # the epilogue for qc emits right after ctx(qc, NKT-1).
        ctxu = {}        # qc -> [4 PSUM accumulators]
        evq = {}         # (qc, kt) -> [e_ft0, e_ft1]

        def emit_scores(qc, kt):
            qsl = slice(qc * QC, (qc + 1) * QC)
            ksl = slice(kt * P, (kt + 1) * P)
            evq[(qc, kt)] = []
            for ft in range(FT):
                sc = ps_sc.tile([P, 2 * QC], f32, tag="sc", name="sc")
                nc.tensor.matmul(
                    sc[:, 0:QC],
                    lhsT=KT[0:64, ft, ksl],
                    rhs=QT[0:64, ft, qsl],
                    start=True, stop=True,
                    tile_position=(0, 0),
                )
                nc.tensor.matmul(
                    sc[:, QC:2 * QC],
                    lhsT=KT[64:128, ft, ksl],
                    rhs=QT[64:128, ft, qsl],
                    start=True, stop=True,
                    tile_position=(64, 0),
                )
                e = exps.tile([P, 2 * QC], bf16, tag="exps", name="e")
                nc.scalar.activation(e, sc, EXP)
                evq[(qc, kt)].append(e)

        def emit_ctx(qc, kt):
            first, last = kt == 0, kt == NKT - 1
            if first:
                ctxu[qc] = [
                    ps_acc.tile([P, QC], f32, tag="acc", name=f"ctxu{hh}")
                    for hh in range(HG)
                ]
            ex = evq.pop((qc, kt))
            for h in range(HG):
                # rows 0:64 = unnormalized ctx^T dims, row 64 = denominator
                nc.tensor.matmul(
                    ctxu[qc][h][0:65, :],
                    lhsT=V[:, kt, h, 0:65],
                    rhs=ex[h // 2][:, (h % 2) * QC:(h % 2 + 1) * QC],
                    start=first, stop=last,
                )

        cts = {}         # qc -> normalized per-head ct tile

        def emit_epilogue_a(qc):
            # Normalize: reciprocal of row-64 denominators (DVE), broadcast
            # across 64 partitions via a K=1 ones-matmul (PE -> borrowed
            # score-PSUM banks), copy to SBUF (DVE), multiply (DVE).
            cu = ctxu.pop(qc)
            recip = small.tile([P, HG, QC], bf16, tag="recip", name="recip")
            bc = bcast.tile([64, HG, QC], f32, tag="bcast", name="bc")
            ct4 = ctpool.tile([64, HG, QC], bf16, tag="ct", name="ct4")
            bct = [
                ps_sc.tile([P, 2 * QC], f32, tag="sc", name="bct0"),
                ps_sc.tile([P, 2 * QC], f32, tag="sc", name="bct1"),
            ]
            for h in range(HG):
                nc.vector.reciprocal(recip[64:65, h, :], cu[h][64:65, :])
                t, col = divmod(h, 2)
                nc.tensor.matmul(
                    bct[t][0:64, col * QC:(col + 1) * QC],
                    lhsT=ones_sb[64:65, :],
                    rhs=recip[64:65, h, :],
                    start=True, stop=True,
                )
                nc.vector.tensor_copy(
                    out=bc[:, h, :], in_=bct[t][0:64, col * QC:(col + 1) * QC]
                )
                nc.vector.tensor_mul(
                    out=ct4[:, h, :], in0=cu[h][0:64, :], in1=bc[:, h, :]
                )
            cts[qc] = ct4

        def emit_epilogue_b(qc, st):
            # split-K output projection for one 128-row q tile
            ct4 = cts[qc]
            s0 = qc * QC + st * P
            ops = ps_sc.tile([P, 2 * QC], f32, tag="sc", name="ops")
            for jc in range(D // 512):
                for h in range(HG):
                    nc.tensor.matmul(
                        ops[:, jc * QC:(jc + 1) * QC],
                        lhsT=ct4[:, h, st * P:(st + 1) * P],
                        rhs=wo_sb[:, h, jc * 512:(jc + 1) * 512],
                        start=(h == 0),
                        stop=(h == HG - 1),
                    )
            osb = outsb.tile([P, D], f32, tag="osb", name="osb")
            nc.vector.tensor_copy(out=osb[:, :], in_=ops[:, :])
            nc.sync.dma_start(out[s0:s0 + P, :], osb)
            if st == QC // P - 1:
                cts.pop(qc)

        from collections import deque

        pending = deque()
        steps = [(qc, kt) for qc in range(NQC) for kt in range(NKT)]
        for i, (qc, kt) in enumerate(steps):
            emit_scores(qc, kt)
            if pending:
                pending.popleft()()
            if i >= LAG:
                pqc, pkt = steps[i - LAG]
                emit_ctx(pqc, pkt)
                if pkt == NKT - 1:
                    emit_epilogue_a(pqc)
                    for st in range(QC // P):
                        pending.append(
                            lambda pqc=pqc, st=st: emit_epilogue_b(pqc, st)
                        )
        for j in range(len(steps) - LAG, len(steps)):
            pqc, pkt = steps[j]
            emit_ctx(pqc, pkt)
            if pkt == NKT - 1:
                emit_epilogue_a(pqc)
                for st in range(QC // P):
                    pending.append(
                        lambda pqc=pqc, st=st: emit_epilogue_b(pqc, st)
                    )
        while pending:
            pending.popleft()()

    nc.compile()
    return nc


def _get_program():
    global _PROGRAM
    if _PROGRAM is None:
        _PROGRAM = _build_program()
    return _PROGRAM


def _host_shards(q, k, v, Wq, bq, Wk, bk, Wv, bv, Wo, bo):
    """Build the 8 per-core input dicts (host-side transposes/slices)."""
    import ml_dtypes

    b16 = ml_dtypes.bfloat16
    scale = 1.0 / np.sqrt(np.float32(DK))
    qT = [np.ascontiguousarray(q[b].T).astype(b16) for b in range(B)]
    kT = [np.ascontiguousarray(k[b].T).astype(b16) for b in range(B)]
    vT = [np.ascontiguousarray(v[b].T).astype(b16) for b in range(B)]
    in_maps = []
    for c in range(NCORES):
        b, g = divmod(c, NCORES // B)
        fsl = slice(g * FC, (g + 1) * FC)
        in_maps.append({
            "qT": qT[b],
            "kT": kT[b],
            "vT": vT[b],
            "wqT": np.ascontiguousarray(Wq[fsl, :].T * scale).astype(b16),
            "wkT": np.ascontiguousarray(Wk[fsl, :].T).astype(b16),
            "wvT": np.ascontiguousarray(Wv[fsl, :].T).astype(b16),
            "woT": np.ascontiguousarray(Wo[:, fsl].T).astype(b16),
            "bq": np.ascontiguousarray(bq[fsl] * scale),
            "bk": np.ascontiguousarray(bk[fsl]),
        })
    return in_maps


def kernel(q, k, v, mask, Wq, bq, Wk, bk, Wv, bv, Wo, bo):
    q = np.asarray(q, dtype=np.float32)
    k = np.asarray(k, dtype=np.float32)
    v = np.asarray(v, dtype=np.float32)
    mask = np.asarray(mask)
    Wq = np.asarray(Wq, dtype=np.float32)
    bq = np.asarray(bq, dtype=np.float32)
    Wk = np.asarray(Wk, dtype=np.float32)
    bk = np.asarray(bk, dtype=np.float32)
    Wv = np.asarray(Wv, dtype=np.float32)
    bv = np.asarray(bv, dtype=np.float32)
    Wo = np.asarray(Wo, dtype=np.float32)
    bo = np.asarray(bo, dtype=np.float32)

    if not np.all(mask != 0):
        # Unmasked-path kernel; fall back to exact host computation if a
        # nontrivial mask ever shows up (spec fills the mask with ones).
        return _host_reference(q, k, v, mask, Wq, bq, Wk, bk, Wv, bv, Wo, bo)

    from concourse.bass_utils import run_bass_kernel_spmd

    nc = _get_program()
    in_maps = _host_shards(q, k, v, Wq, bq, Wk, bk, Wv, bv, Wo, bo)
    res = run_bass_kernel_spmd(nc, in_maps, core_ids=list(range(NCORES)))

    # host reduction: sum the 4 row-parallel Wo partials per batch,
    # then add the exact bv/bo correction (softmax rows sum to 1).
    const = bv @ Wo.T + bo
    out = np.empty((B, S, D), np.float32)
    gpb = NCORES // B
    for b in range(B):
        acc = res.results[b * gpb]["out"].astype(np.float32)
        for g in range(1, gpb):
            acc = acc + res.results[b * gpb + g]["out"]
        out[b] = acc + const[None, :]
    return out


def _host_reference(q, k, v, mask, Wq, bq, Wk, bk, Wv, bv, Wo, bo):
    def split_heads(x):
        b, s, _ = x.shape
        return x.reshape(b, s, H, DK).transpose(0, 2, 1, 3)

    query = split_heads(q @ Wq.T + bq)
    key_ = split_heads(k @ Wk.T + bk)
    value = split_heads(v @ Wv.T + bv)
    scores = np.einsum("bhqd,bhkd->bhqk", query, key_) / np.sqrt(np.float32(DK))
    scores = np.where(mask == 0, np.float32(-1e9), scores).astype(np.float32)
    scores -= scores.max(axis=-1, keepdims=True)
    e = np.exp(scores)
    attn = e / e.sum(axis=-1, keepdims=True)
    ctx = np.einsum("bhqk,bhkd->bhqd", attn, value)
    ctx = ctx.transpose(0, 2, 1, 3).reshape(q.shape[0], -1, D)
    return (ctx @ Wo.T + bo).astype(np.float32)
